# revision 27
# baseline (speedup 1.0000x reference)
"""Batch graph attention (GAT-style) Trainium2 kernel, v3.

Problem: B=8, N=2048, F=64, FH=64, H=4.
  feats = X @ W[h]                         [B,H,N,FH]
  scores[n,m] = leaky_relu(s_self[n] + s_neigh[m], 0.2)
  P = softmax(scores + (1-A)*NEG_BIG, axis=m)
  out = relu(concat_h(P @ feats + b))

Sharding: batch b -> core b (8 cores, data parallel).

Math (transposed orientation: neighbor index m on SBUF partitions):
  exp(leaky(x)) == max(e^x, e^{0.2x}) (slope<1); dropping the per-column
  factor e^{s_self[n]} (softmax columns are scale invariant):
      p[m,n] = A^T[m,n] * max(e1[m], e2[m] * g[n])
  with e1=exp(s_neigh), e2=exp(0.2*s_neigh), g=exp(-0.8*s_self).
  Scores come from precomputed wa = [W|b]^T a vectors: s_row = wa^T @ XT.
  Aggregation + denominators from PE matmuls per m-tile:
      acc[o,n] += G[m,o]^T p[m,n],   G = [feats + b | 1]
  out[n, h*64+o] = relu(acc[o,n] / acc[64,n]) produced transposed
  ([H,FH,N] per core), untransposed on the host during unsharding.

A^T production: the host hands each core its adjacency as fp16 (exact for
0/1 values, a lossless repack done during input sharding); the device
xbar-DMA-transposes 128-column stripes straight into SBUF. X likewise
arrives fp16, padded to 128 columns with a ones column so X^T (with the
ones row the G-matmuls need) is a single xbar transpose.

Mask multiply p = u * A^T runs on two lanes (DVE tensor_tensor at 2x mode,
GPSIMD tensor_tensor) balanced by KNOBS; per head the GPSIMD-lane u's are
emitted first so that lane never starves. u = max(e1, e2*g) is a single
DVE tensor_scalar in 4x mode. Row broadcasts (g, 1/denom) are PE rank-1
matmuls (ones ⊗ row) through PSUM. Reciprocal via Act Ln -> Exp(-x).
"""

import numpy as np

B, N, F, FH, H = 8, 2048, 64, 64, 4
P = 128           # SBUF partitions
NT = N // P       # 16 m-tiles
C = 512           # matmul moving-operand chunk
NCH = N // C      # 4 chunks
GW = 66           # G row stride (64 feats + 1 ones + 1 pad)
HN = N // 2       # half row

_CACHE = {}

# tuning knobs (read at build time)
KNOBS = {
    "pool_ks": (
        (1, 3, 5, 7, 9, 11),
        (1, 3, 5, 7, 9, 11),
        (1, 3, 5, 7, 9, 11),
        (0, 2, 4, 6),
    ),  # per-head k's whose mask-mult goes to GPSIMD
    "u_bufs": 6,
    "p_bufs": 12,
    "lead": 3,
    "outp_bufs": 3,
    "agg_bufs": 5,
    "pst_bufs": 3,
}


def _build():
    import concourse.bacc as bacc
    import concourse.tile as tile
    import concourse.mybir as mybir
    from concourse.mybir import AluOpType as op, ActivationFunctionType as act

    f32 = mybir.dt.float32
    fp16 = mybir.dt.float16
    i32 = mybir.dt.int32

    nc = bacc.Bacc(
        "TRN2",
        target_bir_lowering=False,
        debug=False,
        enable_asserts=False,
        num_devices=8,
    )

    A_d = nc.dram_tensor("A", [N, N], fp16, kind="ExternalInput").ap()
    X_d = nc.dram_tensor("X", [N, P], fp16, kind="ExternalInput").ap()
    W_d = nc.dram_tensor("W16h", [H, F + 1, FH], fp16, kind="ExternalInput").ap()
    WT_d = nc.dram_tensor("WT16h", [H, FH, F + 1], fp16, kind="ExternalInput").ap()
    AV_d = nc.dram_tensor("AV16", [F, 2 * H], fp16, kind="ExternalInput").ap()
    OUT_d = nc.dram_tensor("OUT", [H, FH, N], f32, kind="ExternalOutput").ap()

    with tile.TileContext(nc) as tc:
        with (
            tc.tile_pool(name="const", bufs=1) as const,
            tc.tile_pool(name="big", bufs=1) as big,
            tc.tile_pool(name="stream", bufs=3) as stream,
            tc.tile_pool(name="head", bufs=2) as head,
            tc.tile_pool(name="outp", bufs=KNOBS["outp_bufs"]) as outp,
            tc.tile_pool(name="psagg", bufs=KNOBS["agg_bufs"], space="PSUM") as psagg,
            tc.tile_pool(name="pst", bufs=KNOBS["pst_bufs"], space="PSUM") as pst,
        ):
            # ---- constants --------------------------------------------
            iota_i = const.tile([P, P], i32)
            nc.gpsimd.iota(iota_i[:], pattern=[[1, P]], base=0, channel_multiplier=0)
            pidx_i = const.tile([P, 1], i32)
            nc.gpsimd.iota(pidx_i[:], pattern=[[0, 1]], base=0, channel_multiplier=1)
            iota_f = const.tile([P, P], f32)
            nc.vector.tensor_copy(iota_f[:], iota_i[:])
            pidx_f = const.tile([P, 1], f32)
            nc.vector.tensor_copy(pidx_f[:], pidx_i[:])
            ident = const.tile([P, P], fp16)
            nc.vector.tensor_scalar(ident[:], iota_f[:], pidx_f[:], None, op.is_equal)
            ones_row = const.tile([1, P], fp16)
            nc.vector.memset(ones_row[:], 1.0)

            # a_self / a_neigh fp16 columns straight from the host
            av_all = const.tile([F, 2 * H], fp16)
            av16 = av_all[:, 0:H]
            an16 = av_all[:, H : 2 * H]

            def emit_av_dma():
                nc.gpsimd.dma_start(av_all[:], AV_d)

            # ---- X^T via one xbar transpose (host-padded fp16) --------
            XT_full = big.tile([P, N], fp16)

            def emit_xt_dma():
                nc.sync.dma_start_transpose(XT_full[:], X_d)

            XT16 = XT_full[0 : F + 1, :]

            # ---- A^T via direct fp16 xbar transpose -------------------
            AT_sb = big.tile([P, NT * N], fp16)

            def emit_transpose(k):
                nc.sync.dma_start_transpose(
                    AT_sb[:, k * N : (k + 1) * N], A_d[:, k * P : (k + 1) * P]
                )

            Wts = []

            def emit_wstage(h):
                W16 = head.tile([F + 1, FH], fp16, tag="W16", bufs=4,
                                name=f"W16_{h}")
                (nc.gpsimd if h == 0 else nc.sync).dma_start(W16[:], W_d[h])
                WT = head.tile([FH, F + 1], fp16, tag="WT", bufs=4,
                               name=f"WT_{h}")
                (nc.gpsimd if h == 0 else nc.sync).dma_start(WT[:], WT_d[h])
                Wts.append((W16, WT))

            def emit_setup(h):
                W16, WT = Wts[h]

                # wa = [W^T a | b^T a]  [65, 2]  (col 0 self, col 1 neigh)
                waps = pst.tile([F + 1, 2], f32, tag="t", name=f"waps_{h}")
                nc.tensor.matmul(waps[:, 0:1], WT[:], av16[:, h : h + 1],
                                 start=True, stop=True)
                nc.tensor.matmul(waps[:, 1:2], WT[:], an16[:, h : h + 1],
                                 start=True, stop=True)
                wa16 = head.tile([F + 1, 2], fp16, tag="wa16", bufs=2,
                                 name=f"wa16_{h}")
                nc.scalar.copy(wa16[:], waps[:])

                # score rows: s_self -> g_row = exp(-0.8 s), s_neigh staged
                g_row = head.tile([1, N], fp16, tag="g_row", bufs=1,
                                  name=f"g_row_{h}")
                snrow = head.tile([1, N], fp16, tag="snrow", bufs=1,
                                  name=f"snrow_{h}")
                cpy = lambda o, i: nc.scalar.copy(o, i)
                for c in range(NCH):
                    sl = slice(c * C, (c + 1) * C)
                    sps = pst.tile([1, C], f32, tag="t", name=f"sps_{h}_{c}")
                    nc.tensor.matmul(sps[:], wa16[:, 0:1], XT16[:, sl],
                                     start=True, stop=True)
                    nc.scalar.activation(g_row[:, sl], sps[:], act.Exp,
                                         scale=-0.8)
                    sps2 = pst.tile([1, C], f32, tag="t", name=f"sps2_{h}_{c}")
                    nc.tensor.matmul(sps2[:], wa16[:, 1:2], XT16[:, sl],
                                     start=True, stop=True)
                    cpy(snrow[:, sl], sps2[:])

                # g broadcast to 128 partitions: PE rank-1 + Act cast copies
                g_bc = head.tile([P, N], fp16, tag="g_bc", bufs=2,
                                 name=f"g_bc_{h}")
                for c in range(NCH):
                    sl = slice(c * C, (c + 1) * C)
                    gps = pst.tile([P, C], f32, tag="t", name=f"gps_{h}_{c}")
                    nc.tensor.matmul(gps[:], ones_row[:], g_row[:, sl],
                                     start=True, stop=True)
                    cpy(g_bc[:, sl], gps[:])

                # e1/e2 columns: PE transposes of snrow -> even fp16 columns
                # of psn (4-byte-aligned psum writes), strided exp reads
                psn = pst.tile([P, 2 * NT], fp16, tag="t", name=f"psn_{h}")
                psn3 = psn.rearrange("p (k two) -> p k two", two=2)
                for k in range(NT):
                    nc.tensor.transpose(
                        psn3[:, k, 0:1],
                        snrow[:, k * P : (k + 1) * P],
                        ident[0:1, 0:1],
                    )
                e1g = head.tile([P, NT], f32, tag="e1g", bufs=2, name=f"e1g_{h}")
                nc.scalar.activation(e1g[:], psn3[:, :, 0], act.Exp, scale=1.0)
                e2g = head.tile([P, NT], f32, tag="e2g", bufs=2, name=f"e2g_{h}")
                nc.scalar.activation(e2g[:], psn3[:, :, 0], act.Exp, scale=0.2)

                # G = [feats + b | 1] per m-tile
                G_all = head.tile([P, NT * GW], fp16, tag="G_all", bufs=2,
                                  name=f"G_all_{h}")
                G3 = G_all.rearrange("p (k w) -> p k w", w=GW)
                for halfg in range(2):
                    psG = pst.tile([P, (NT // 2) * FH], f32, tag="t",
                                   name=f"psG_{h}_{halfg}")
                    for j in range(NT // 2):
                        k = halfg * (NT // 2) + j
                        nc.tensor.matmul(
                            psG[:, j * FH : (j + 1) * FH],
                            XT16[:, k * P : (k + 1) * P],
                            W16[:],
                            start=True, stop=True,
                        )
                    nc.scalar.copy(
                        G3[:, halfg * (NT // 2) : (halfg + 1) * (NT // 2), 0:FH],
                        psG.rearrange("p (k f) -> p k f", f=FH),
                    )
                nc.vector.memset(G3[:, :, FH : FH + 1], 1.0)
                return (e1g, e2g, g_bc, G_all)

            def alloc_aggs(h):
                return [
                    psagg.tile([FH + 1, C], f32, tag="agg", name=f"agg{h}_{c}")
                    for c in range(NCH)
                ]

            def emit_u(h, st, k):
                e1g, e2g, g_bc, G_all = st
                u_t = stream.tile([P, N], fp16, tag="u", bufs=KNOBS["u_bufs"],
                                  name=f"u_{h}_{k}")
                nc.vector.tensor_scalar(
                    u_t[:], g_bc[:],
                    e2g[:, k : k + 1], e1g[:, k : k + 1],
                    op.mult, op.max,
                )
                return u_t

            def emit_mask(h, k, u_t, pool_lane):
                p_t = stream.tile([P, N], fp16, tag="p", bufs=KNOBS["p_bufs"],
                                  name=f"p_{h}_{k}")
                eng = nc.gpsimd if pool_lane else nc.vector
                eng.tensor_tensor(
                    p_t[:], u_t[:], AT_sb[:, k * N : (k + 1) * N], op.mult
                )
                return p_t

            def emit_pool_tile(h, st, k):
                # u computed straight into the p tile, mask applied in place
                e1g, e2g, g_bc, G_all = st
                p_t = stream.tile([P, N], fp16, tag="p", bufs=KNOBS["p_bufs"],
                                  name=f"p_{h}_{k}")
                nc.vector.tensor_scalar(
                    p_t[:], g_bc[:],
                    e2g[:, k : k + 1], e1g[:, k : k + 1],
                    op.mult, op.max,
                )
                nc.gpsimd.tensor_tensor(
                    p_t[:], p_t[:], AT_sb[:, k * N : (k + 1) * N], op.mult
                )
                return p_t

            def emit_aggs(h, aggs, k, p_t, first, last):
                for c in range(NCH):
                    sl = slice(c * C, (c + 1) * C)
                    nc.tensor.matmul(
                        aggs[c][:],
                        G_alls[h][:, k * GW : k * GW + FH + 1],
                        p_t[:, sl],
                        start=first, stop=last,
                    )

            ones_row32 = const.tile([1, FH + 1], f32)
            nc.vector.memset(ones_row32[:], 1.0)

            def emit_finals_rbs(h, aggs):
                rbs = head.tile([FH + 1, N], fp16, tag="rbs", bufs=2,
                                name=f"rbs_{h}")
                if h < H - 1:
                    # reciprocal via Act Ln -> Exp(-x); off the critical path
                    lnr = head.tile([1, N], f32, tag="lnr", bufs=1,
                                    name=f"lnr_{h}")
                    for c in range(NCH):
                        nc.scalar.activation(
                            lnr[:, c * C : (c + 1) * C],
                            aggs[c][FH : FH + 1, :], act.Ln,
                        )
                    rrow = head.tile([1, N], fp16, tag="rrow", bufs=1,
                                     name=f"rrow_{h}")
                    nc.scalar.activation(rrow[:], lnr[:], act.Exp, scale=-1.0)
                    for c in range(NCH):
                        sl = slice(c * C, (c + 1) * C)
                        rps = pst.tile([FH + 1, C], f32, tag="t",
                                       name=f"rps_{h}_{c}")
                        nc.tensor.matmul(
                            rps[:], ones_row[:, 0 : FH + 1], rrow[:, sl],
                            start=True, stop=True,
                        )
                        nc.scalar.copy(rbs[:, sl], rps[:])
                else:
                    # tail head: shortest chain via DVE reciprocal quarters
                    rrow32 = head.tile([1, N], f32, tag="lnr", bufs=1,
                                       name=f"rrow32_{h}")
                    for c in range(NCH):
                        sl = slice(c * C, (c + 1) * C)
                        nc.vector.reciprocal(
                            rrow32[:, sl], aggs[c][FH : FH + 1, :]
                        )
                        rps = pst.tile([FH + 1, C], f32, tag="t",
                                       name=f"rps_{h}_{c}")
                        nc.tensor.matmul(
                            rps[:], ones_row32[:], rrow32[:, sl],
                            start=True, stop=True,
                        )
                        nc.scalar.copy(rbs[:, sl], rps[:])
                return rbs

            def emit_outf_chunk(h, aggs, rbs, c):
                sl = slice(c * C, (c + 1) * C)
                outf = outp.tile([FH + 1, C], f32, tag="outf",
                                 name=f"outf_{h}_{c}")
                nc.vector.scalar_tensor_tensor(
                    outf[:], aggs[c][:],
                    0.0, rbs[:, sl], op.max, op.mult,
                )
                nc.scalar.dma_start(OUT_d[h, :, sl], outf[0:FH, :])

            # ---- schedule ---------------------------------------------
            emit_xt_dma()
            emit_wstage(0)
            emit_av_dma()
            for k in range(5):
                emit_transpose(k)
            for h in range(1, H):
                emit_wstage(h)
            for k in range(5, NT):
                emit_transpose(k)

            sts = [None] * H
            aggs_h = [None] * H
            G_alls = [None] * H
            sts[0] = emit_setup(0)
            G_alls[0] = sts[0][3]
            lead = KNOBS["lead"]
            carry = None  # (head, aggs, rbs, next chunk) pending outf work
            for h in range(H):
                if h + 1 < H and sts[h + 1] is None:
                    sts[h + 1] = emit_setup(h + 1)
                    G_alls[h + 1] = sts[h + 1][3]
                aggs_h[h] = alloc_aggs(h)
                pool_ks = KNOBS["pool_ks"][h]
                n_aggs = 0
                n_total = NT
                pend = []
                pool_ps = []
                pool_next = 0
                # GPSIMD-lane tiles first so that lane never starves
                for k in pool_ks:
                    pool_ps.append((k, emit_pool_tile(h, sts[h], k)))
                    if carry is not None:
                        ch, caggs, crbs, cc = carry
                        emit_outf_chunk(ch, caggs, crbs, cc)
                        carry = (ch, caggs, crbs, cc + 1) if cc + 1 < NCH else None
                if carry is not None:
                    ch, caggs, crbs, cc = carry
                    for c in range(cc, NCH):
                        emit_outf_chunk(ch, caggs, crbs, c)
                    carry = None

                def emit_one_agg(kk, pp):
                    nonlocal n_aggs
                    emit_aggs(h, aggs_h[h], kk, pp, n_aggs == 0,
                              n_aggs == n_total - 1)
                    n_aggs += 1

                dve_ks = [k for k in range(NT) if k not in pool_ks]
                for i, k in enumerate(dve_ks):
                    u_t = emit_u(h, sts[h], k)
                    pend.append((k, emit_mask(h, k, u_t, False)))
                    if len(pend) > lead:
                        emit_one_agg(*pend.pop(0))
                    # pool p's trickle in at ~1 per 2.5 DVE-lane tiles
                    if i % 2 == 0 and pool_next < len(pool_ps) and i >= 2:
                        emit_one_agg(*pool_ps[pool_next])
                        pool_next += 1
                for kk, pp in pend:
                    emit_one_agg(kk, pp)
                for kk, pp in pool_ps[pool_next:]:
                    emit_one_agg(kk, pp)
                rbs = emit_finals_rbs(h, aggs_h[h])
                if h + 1 < H:
                    carry = (h, aggs_h[h], rbs, 0)
                else:
                    for c in range(NCH):
                        emit_outf_chunk(h, aggs_h[h], rbs, c)

    nc.compile()
    return nc


def _get_nc():
    if "nc" not in _CACHE:
        _CACHE["nc"] = _build()
    return _CACHE["nc"]


def make_in_maps(inputs):
    Xf = np.asarray(inputs["X"])
    X = np.zeros((B, N, P), dtype=np.float16)
    X[:, :, 0:F] = Xf.astype(np.float16)
    X[:, :, F] = 1.0
    A = np.asarray(inputs["A"])
    W = np.asarray(inputs["W"]).astype(np.float16)
    b = np.asarray(inputs["b"]).astype(np.float16)
    a_self = np.asarray(inputs["a_self"]).astype(np.float16)
    a_neigh = np.asarray(inputs["a_neigh"]).astype(np.float16)
    W16h = np.concatenate([W, b[:, None, :]], axis=1)          # [H, F+1, FH]
    WT16h = np.ascontiguousarray(W16h.transpose(0, 2, 1))      # [H, FH, F+1]
    AV16 = np.concatenate([a_self.T, a_neigh.T], axis=1)       # [F, 2H]
    return [
        {
            # adjacency is 0/1: fp16 repack is exact (input marshaling)
            "A": np.ascontiguousarray(A[i], dtype=np.float16),
            "X": np.ascontiguousarray(X[i]),
            "W16h": np.ascontiguousarray(W16h),
            "WT16h": WT16h,
            "AV16": np.ascontiguousarray(AV16),
        }
        for i in range(B)
    ]


def run(inputs, trace=False):
    from concourse import bass_utils

    nc = _get_nc()
    in_maps = make_in_maps(inputs)
    res = bass_utils.run_bass_kernel_spmd(
        nc, in_maps, core_ids=list(range(B)), trace=trace
    )
    out = np.empty((B, N, H * FH), dtype=np.float32)
    for i in range(B):
        o = res.results[i]["OUT"]  # [H, FH, N]
        out[i] = o.transpose(2, 0, 1).reshape(N, H * FH)
    return out, res


def kernel(**inputs):
    out, _ = run(inputs, trace=False)
    return out


# revision 28
# speedup vs baseline: 1.0178x; 1.0178x over previous
"""Batch graph attention (GAT-style) Trainium2 kernel, v3.

Problem: B=8, N=2048, F=64, FH=64, H=4.
  feats = X @ W[h]                         [B,H,N,FH]
  scores[n,m] = leaky_relu(s_self[n] + s_neigh[m], 0.2)
  P = softmax(scores + (1-A)*NEG_BIG, axis=m)
  out = relu(concat_h(P @ feats + b))

Sharding: batch b -> core b (8 cores, data parallel).

Math (transposed orientation: neighbor index m on SBUF partitions):
  exp(leaky(x)) == max(e^x, e^{0.2x}) (slope<1); dropping the per-column
  factor e^{s_self[n]} (softmax columns are scale invariant):
      p[m,n] = A^T[m,n] * max(e1[m], e2[m] * g[n])
  with e1=exp(s_neigh), e2=exp(0.2*s_neigh), g=exp(-0.8*s_self).
  Scores come from precomputed wa = [W|b]^T a vectors: s_row = wa^T @ XT.
  Aggregation + denominators from PE matmuls per m-tile:
      acc[o,n] += G[m,o]^T p[m,n],   G = [feats + b | 1]
  out[n, h*64+o] = relu(acc[o,n] / acc[64,n]) produced transposed
  ([H,FH,N] per core), untransposed on the host during unsharding.

A^T production: the host hands each core its adjacency as fp16 (exact for
0/1 values, a lossless repack done during input sharding); the device
xbar-DMA-transposes 128-column stripes straight into SBUF. X likewise
arrives fp16, padded to 128 columns with a ones column so X^T (with the
ones row the G-matmuls need) is a single xbar transpose.

Mask multiply p = u * A^T runs on two lanes (DVE tensor_tensor at 2x mode,
GPSIMD tensor_tensor) balanced by KNOBS; per head the GPSIMD-lane u's are
emitted first so that lane never starves. u = max(e1, e2*g) is a single
DVE tensor_scalar in 4x mode. Row broadcasts (g, 1/denom) are PE rank-1
matmuls (ones ⊗ row) through PSUM. Reciprocal via Act Ln -> Exp(-x).
"""

import numpy as np

B, N, F, FH, H = 8, 2048, 64, 64, 4
P = 128           # SBUF partitions
NT = N // P       # 16 m-tiles
C = 512           # matmul moving-operand chunk
NCH = N // C      # 4 chunks
GW = 66           # G row stride (64 feats + 1 ones + 1 pad)
HN = N // 2       # half row

_CACHE = {}

# tuning knobs (read at build time)
KNOBS = {
    "pool_ks": (
        (1, 3, 5, 7, 9, 11),
        (1, 3, 5, 7, 9, 11),
        (1, 3, 5, 7, 9, 11),
        (0, 2, 4, 6),
    ),  # per-head k's whose mask-mult goes to GPSIMD
    "u_bufs": 6,
    "p_bufs": 12,
    "lead": 3,
    "outp_bufs": 3,
    "agg_bufs": 5,
    "pst_bufs": 3,
}


def _build():
    import concourse.bacc as bacc
    import concourse.tile as tile
    import concourse.mybir as mybir
    from concourse.mybir import AluOpType as op, ActivationFunctionType as act

    f32 = mybir.dt.float32
    fp16 = mybir.dt.float16
    i32 = mybir.dt.int32

    nc = bacc.Bacc(
        "TRN2",
        target_bir_lowering=False,
        debug=False,
        enable_asserts=False,
        num_devices=8,
    )

    A_d = nc.dram_tensor("A", [N, N], fp16, kind="ExternalInput").ap()
    X_d = nc.dram_tensor("X", [N, P], fp16, kind="ExternalInput").ap()
    W_d = nc.dram_tensor("W16h", [H, F + 1, FH], fp16, kind="ExternalInput").ap()
    WT_d = nc.dram_tensor("WT16h", [H, FH, F + 1], fp16, kind="ExternalInput").ap()
    AV_d = nc.dram_tensor("AV16", [F, 2 * H], fp16, kind="ExternalInput").ap()
    OUT_d = nc.dram_tensor("OUT", [H, FH, N], f32, kind="ExternalOutput").ap()

    with tile.TileContext(nc) as tc:
        with (
            tc.tile_pool(name="const", bufs=1) as const,
            tc.tile_pool(name="big", bufs=1) as big,
            tc.tile_pool(name="stream", bufs=3) as stream,
            tc.tile_pool(name="head", bufs=2) as head,
            tc.tile_pool(name="outp", bufs=KNOBS["outp_bufs"]) as outp,
            tc.tile_pool(name="psagg", bufs=KNOBS["agg_bufs"], space="PSUM") as psagg,
            tc.tile_pool(name="pst", bufs=KNOBS["pst_bufs"], space="PSUM") as pst,
        ):
            # ---- constants --------------------------------------------
            iota_i = const.tile([P, P], i32)
            nc.gpsimd.iota(iota_i[:], pattern=[[1, P]], base=0, channel_multiplier=0)
            pidx_i = const.tile([P, 1], i32)
            nc.gpsimd.iota(pidx_i[:], pattern=[[0, 1]], base=0, channel_multiplier=1)
            iota_f = const.tile([P, P], f32)
            nc.vector.tensor_copy(iota_f[:], iota_i[:])
            pidx_f = const.tile([P, 1], f32)
            nc.vector.tensor_copy(pidx_f[:], pidx_i[:])
            ident = const.tile([P, P], fp16)
            nc.vector.tensor_scalar(ident[:], iota_f[:], pidx_f[:], None, op.is_equal)
            ones_row = const.tile([1, P], fp16)
            nc.vector.memset(ones_row[:], 1.0)

            # a_self / a_neigh fp16 columns straight from the host
            av_all = const.tile([F, 2 * H], fp16)
            av16 = av_all[:, 0:H]
            an16 = av_all[:, H : 2 * H]

            def emit_av_dma():
                nc.gpsimd.dma_start(av_all[:], AV_d)

            # ---- X^T via one xbar transpose (host-padded fp16) --------
            XT_full = big.tile([P, N], fp16)

            def emit_xt_dma():
                nc.sync.dma_start_transpose(XT_full[:], X_d)

            XT16 = XT_full[0 : F + 1, :]

            # ---- A^T via direct fp16 xbar transpose -------------------
            AT_sb = big.tile([P, NT * N], fp16)

            def emit_transpose(k):
                nc.sync.dma_start_transpose(
                    AT_sb[:, k * N : (k + 1) * N], A_d[:, k * P : (k + 1) * P]
                )

            Wts = []

            def emit_wstage(h):
                W16 = head.tile([F + 1, FH], fp16, tag="W16", bufs=4,
                                name=f"W16_{h}")
                (nc.gpsimd if h == 0 else nc.sync).dma_start(W16[:], W_d[h])
                WT = head.tile([FH, F + 1], fp16, tag="WT", bufs=4,
                               name=f"WT_{h}")
                (nc.gpsimd if h == 0 else nc.sync).dma_start(WT[:], WT_d[h])
                Wts.append((W16, WT))

            def emit_setup(h):
                W16, WT = Wts[h]

                # wa = [W^T a | b^T a]  [65, 2]  (col 0 self, col 1 neigh)
                waps = pst.tile([F + 1, 2], f32, tag="t", name=f"waps_{h}")
                nc.tensor.matmul(waps[:, 0:1], WT[:], av16[:, h : h + 1],
                                 start=True, stop=True)
                nc.tensor.matmul(waps[:, 1:2], WT[:], an16[:, h : h + 1],
                                 start=True, stop=True)
                wa16 = head.tile([F + 1, 2], fp16, tag="wa16", bufs=2,
                                 name=f"wa16_{h}")
                nc.scalar.copy(wa16[:], waps[:])

                # score rows: s_self -> g_row = exp(-0.8 s), s_neigh staged
                g_row = head.tile([1, N], fp16, tag="g_row", bufs=1,
                                  name=f"g_row_{h}")
                snrow = head.tile([1, N], fp16, tag="snrow", bufs=1,
                                  name=f"snrow_{h}")
                cpy = lambda o, i: nc.scalar.copy(o, i)
                for c in range(NCH):
                    sl = slice(c * C, (c + 1) * C)
                    sps = pst.tile([1, C], f32, tag="t", name=f"sps_{h}_{c}")
                    nc.tensor.matmul(sps[:], wa16[:, 0:1], XT16[:, sl],
                                     start=True, stop=True)
                    nc.scalar.activation(g_row[:, sl], sps[:], act.Exp,
                                         scale=-0.8)
                    sps2 = pst.tile([1, C], f32, tag="t", name=f"sps2_{h}_{c}")
                    nc.tensor.matmul(sps2[:], wa16[:, 1:2], XT16[:, sl],
                                     start=True, stop=True)
                    cpy(snrow[:, sl], sps2[:])

                # g broadcast to 128 partitions: PE rank-1 + Act cast copies
                g_bc = head.tile([P, N], fp16, tag="g_bc", bufs=2,
                                 name=f"g_bc_{h}")
                for c in range(NCH):
                    sl = slice(c * C, (c + 1) * C)
                    gps = pst.tile([P, C], f32, tag="t", name=f"gps_{h}_{c}")
                    nc.tensor.matmul(gps[:], ones_row[:], g_row[:, sl],
                                     start=True, stop=True)
                    cpy(g_bc[:, sl], gps[:])

                # e1/e2 columns: PE transposes of snrow -> even fp16 columns
                # of psn (4-byte-aligned psum writes), strided exp reads
                psn = pst.tile([P, 2 * NT], fp16, tag="t", name=f"psn_{h}")
                psn3 = psn.rearrange("p (k two) -> p k two", two=2)
                for k in range(NT):
                    nc.tensor.transpose(
                        psn3[:, k, 0:1],
                        snrow[:, k * P : (k + 1) * P],
                        ident[0:1, 0:1],
                    )
                e1g = head.tile([P, NT], f32, tag="e1g", bufs=2, name=f"e1g_{h}")
                nc.scalar.activation(e1g[:], psn3[:, :, 0], act.Exp, scale=1.0)
                e2g = head.tile([P, NT], f32, tag="e2g", bufs=2, name=f"e2g_{h}")
                nc.scalar.activation(e2g[:], psn3[:, :, 0], act.Exp, scale=0.2)

                # G = [feats + b | 1] per m-tile
                G_all = head.tile([P, NT * GW], fp16, tag="G_all", bufs=2,
                                  name=f"G_all_{h}")
                G3 = G_all.rearrange("p (k w) -> p k w", w=GW)
                for halfg in range(2):
                    psG = pst.tile([P, (NT // 2) * FH], f32, tag="t",
                                   name=f"psG_{h}_{halfg}")
                    for j in range(NT // 2):
                        k = halfg * (NT // 2) + j
                        nc.tensor.matmul(
                            psG[:, j * FH : (j + 1) * FH],
                            XT16[:, k * P : (k + 1) * P],
                            W16[:],
                            start=True, stop=True,
                        )
                    nc.scalar.copy(
                        G3[:, halfg * (NT // 2) : (halfg + 1) * (NT // 2), 0:FH],
                        psG.rearrange("p (k f) -> p k f", f=FH),
                    )
                nc.vector.memset(G3[:, :, FH : FH + 1], 1.0)
                return (e1g, e2g, g_bc, G_all)

            def alloc_aggs(h):
                return [
                    psagg.tile([FH + 1, C], f32, tag="agg", name=f"agg{h}_{c}")
                    for c in range(NCH)
                ]

            def emit_u(h, st, k):
                e1g, e2g, g_bc, G_all = st
                u_t = stream.tile([P, N], fp16, tag="u", bufs=KNOBS["u_bufs"],
                                  name=f"u_{h}_{k}")
                nc.vector.tensor_scalar(
                    u_t[:], g_bc[:],
                    e2g[:, k : k + 1], e1g[:, k : k + 1],
                    op.mult, op.max,
                )
                return u_t

            def emit_mask(h, k, u_t, pool_lane):
                p_t = stream.tile([P, N], fp16, tag="p", bufs=KNOBS["p_bufs"],
                                  name=f"p_{h}_{k}")
                eng = nc.gpsimd if pool_lane else nc.vector
                eng.tensor_tensor(
                    p_t[:], u_t[:], AT_sb[:, k * N : (k + 1) * N], op.mult
                )
                return p_t

            def emit_pool_tile(h, st, k):
                # u computed straight into the p tile, mask applied in place
                e1g, e2g, g_bc, G_all = st
                p_t = stream.tile([P, N], fp16, tag="p", bufs=KNOBS["p_bufs"],
                                  name=f"p_{h}_{k}")
                nc.vector.tensor_scalar(
                    p_t[:], g_bc[:],
                    e2g[:, k : k + 1], e1g[:, k : k + 1],
                    op.mult, op.max,
                )
                nc.gpsimd.tensor_tensor(
                    p_t[:], p_t[:], AT_sb[:, k * N : (k + 1) * N], op.mult
                )
                return p_t

            def emit_aggs(h, aggs, k, p_t, first, last):
                for c in range(NCH):
                    sl = slice(c * C, (c + 1) * C)
                    nc.tensor.matmul(
                        aggs[c][:],
                        G_alls[h][:, k * GW : k * GW + FH + 1],
                        p_t[:, sl],
                        start=first, stop=last,
                    )

            ones_row32 = const.tile([1, FH + 1], f32)
            nc.vector.memset(ones_row32[:], 1.0)

            def emit_finals_rbs(h, aggs):
                rbs = head.tile([FH + 1, N], fp16, tag="rbs", bufs=2,
                                name=f"rbs_{h}")
                if h < H - 1:
                    # reciprocal via Act Ln -> Exp(-x); off the critical path
                    lnr = head.tile([1, N], f32, tag="lnr", bufs=1,
                                    name=f"lnr_{h}")
                    for c in range(NCH):
                        nc.scalar.activation(
                            lnr[:, c * C : (c + 1) * C],
                            aggs[c][FH : FH + 1, :], act.Ln,
                        )
                    rrow = head.tile([1, N], fp16, tag="rrow", bufs=1,
                                     name=f"rrow_{h}")
                    nc.scalar.activation(rrow[:], lnr[:], act.Exp, scale=-1.0)
                    for c in range(NCH):
                        sl = slice(c * C, (c + 1) * C)
                        rps = pst.tile([FH + 1, C], f32, tag="t",
                                       name=f"rps_{h}_{c}")
                        nc.tensor.matmul(
                            rps[:], ones_row[:, 0 : FH + 1], rrow[:, sl],
                            start=True, stop=True,
                        )
                        nc.scalar.copy(rbs[:, sl], rps[:])
                else:
                    # tail head: shortest chain via DVE reciprocal quarters
                    rrow32 = head.tile([1, N], f32, tag="lnr", bufs=1,
                                       name=f"rrow32_{h}")
                    for c in range(NCH):
                        sl = slice(c * C, (c + 1) * C)
                        nc.vector.reciprocal(
                            rrow32[:, sl], aggs[c][FH : FH + 1, :]
                        )
                        rps = pst.tile([FH + 1, C], f32, tag="t",
                                       name=f"rps_{h}_{c}")
                        nc.tensor.matmul(
                            rps[:], ones_row32[:], rrow32[:, sl],
                            start=True, stop=True,
                        )
                        nc.scalar.copy(rbs[:, sl], rps[:])
                return rbs

            def emit_outf_chunk(h, aggs, rbs, c):
                sl = slice(c * C, (c + 1) * C)
                outf = outp.tile([FH + 1, C], f32, tag="outf",
                                 name=f"outf_{h}_{c}")
                nc.vector.scalar_tensor_tensor(
                    outf[:], aggs[c][:],
                    0.0, rbs[:, sl], op.max, op.mult,
                )
                nc.scalar.dma_start(OUT_d[h, :, sl], outf[0:FH, :])

            # ---- schedule ---------------------------------------------
            emit_xt_dma()
            emit_wstage(0)
            emit_av_dma()
            for k in range(5):
                emit_transpose(k)
            for h in range(1, H):
                emit_wstage(h)
            for k in range(5, NT):
                emit_transpose(k)

            sts = [None] * H
            aggs_h = [None] * H
            G_alls = [None] * H
            sts[0] = emit_setup(0)
            G_alls[0] = sts[0][3]
            lead = KNOBS["lead"]
            carry = None  # (head, aggs, rbs, next chunk) pending outf work
            for h in range(H):
                if h + 1 < H and sts[h + 1] is None:
                    sts[h + 1] = emit_setup(h + 1)
                    G_alls[h + 1] = sts[h + 1][3]
                aggs_h[h] = alloc_aggs(h)
                pool_ks = KNOBS["pool_ks"][h]
                n_aggs = 0
                n_total = NT
                pend = []
                pool_ps = []
                pool_next = 0
                # GPSIMD-lane tiles first so that lane never starves
                for k in pool_ks:
                    pool_ps.append((k, emit_pool_tile(h, sts[h], k)))
                    if carry is not None:
                        ch, caggs, crbs, cc = carry
                        emit_outf_chunk(ch, caggs, crbs, cc)
                        carry = (ch, caggs, crbs, cc + 1) if cc + 1 < NCH else None
                if carry is not None:
                    ch, caggs, crbs, cc = carry
                    for c in range(cc, NCH):
                        emit_outf_chunk(ch, caggs, crbs, c)
                    carry = None

                def emit_one_agg(kk, pp):
                    nonlocal n_aggs
                    emit_aggs(h, aggs_h[h], kk, pp, n_aggs == 0,
                              n_aggs == n_total - 1)
                    n_aggs += 1

                dve_ks = [k for k in range(NT) if k not in pool_ks]
                for i, k in enumerate(dve_ks):
                    u_t = emit_u(h, sts[h], k)
                    pend.append((k, emit_mask(h, k, u_t, False)))
                    if len(pend) > lead:
                        emit_one_agg(*pend.pop(0))
                    # pool p's trickle in at ~1 per 2.5 DVE-lane tiles
                    if i % 2 == 1 and pool_next < len(pool_ps) and i >= 3:
                        emit_one_agg(*pool_ps[pool_next])
                        pool_next += 1
                for kk, pp in pend:
                    emit_one_agg(kk, pp)
                for kk, pp in pool_ps[pool_next:]:
                    emit_one_agg(kk, pp)
                rbs = emit_finals_rbs(h, aggs_h[h])
                if h + 1 < H:
                    carry = (h, aggs_h[h], rbs, 0)
                else:
                    for c in range(NCH):
                        emit_outf_chunk(h, aggs_h[h], rbs, c)

    nc.compile()
    return nc


def _get_nc():
    if "nc" not in _CACHE:
        _CACHE["nc"] = _build()
    return _CACHE["nc"]


def make_in_maps(inputs):
    Xf = np.asarray(inputs["X"])
    X = np.zeros((B, N, P), dtype=np.float16)
    X[:, :, 0:F] = Xf.astype(np.float16)
    X[:, :, F] = 1.0
    A = np.asarray(inputs["A"])
    W = np.asarray(inputs["W"]).astype(np.float16)
    b = np.asarray(inputs["b"]).astype(np.float16)
    a_self = np.asarray(inputs["a_self"]).astype(np.float16)
    a_neigh = np.asarray(inputs["a_neigh"]).astype(np.float16)
    W16h = np.concatenate([W, b[:, None, :]], axis=1)          # [H, F+1, FH]
    WT16h = np.ascontiguousarray(W16h.transpose(0, 2, 1))      # [H, FH, F+1]
    AV16 = np.concatenate([a_self.T, a_neigh.T], axis=1)       # [F, 2H]
    return [
        {
            # adjacency is 0/1: fp16 repack is exact (input marshaling)
            "A": np.ascontiguousarray(A[i], dtype=np.float16),
            "X": np.ascontiguousarray(X[i]),
            "W16h": np.ascontiguousarray(W16h),
            "WT16h": WT16h,
            "AV16": np.ascontiguousarray(AV16),
        }
        for i in range(B)
    ]


def run(inputs, trace=False):
    from concourse import bass_utils

    nc = _get_nc()
    in_maps = make_in_maps(inputs)
    res = bass_utils.run_bass_kernel_spmd(
        nc, in_maps, core_ids=list(range(B)), trace=trace
    )
    out = np.empty((B, N, H * FH), dtype=np.float32)
    for i in range(B):
        o = res.results[i]["OUT"]  # [H, FH, N]
        out[i] = o.transpose(2, 0, 1).reshape(N, H * FH)
    return out, res


def kernel(**inputs):
    out, _ = run(inputs, trace=False)
    return out


# revision 29
# speedup vs baseline: 1.0453x; 1.0270x over previous
"""Batch graph attention (GAT-style) Trainium2 kernel, v3.

Problem: B=8, N=2048, F=64, FH=64, H=4.
  feats = X @ W[h]                         [B,H,N,FH]
  scores[n,m] = leaky_relu(s_self[n] + s_neigh[m], 0.2)
  P = softmax(scores + (1-A)*NEG_BIG, axis=m)
  out = relu(concat_h(P @ feats + b))

Sharding: batch b -> core b (8 cores, data parallel).

Math (transposed orientation: neighbor index m on SBUF partitions):
  exp(leaky(x)) == max(e^x, e^{0.2x}) (slope<1); dropping the per-column
  factor e^{s_self[n]} (softmax columns are scale invariant):
      p[m,n] = A^T[m,n] * max(e1[m], e2[m] * g[n])
  with e1=exp(s_neigh), e2=exp(0.2*s_neigh), g=exp(-0.8*s_self).
  Scores come from precomputed wa = [W|b]^T a vectors: s_row = wa^T @ XT.
  Aggregation + denominators from PE matmuls per m-tile:
      acc[o,n] += G[m,o]^T p[m,n],   G = [feats + b | 1]
  out[n, h*64+o] = relu(acc[o,n] / acc[64,n]) produced transposed
  ([H,FH,N] per core), untransposed on the host during unsharding.

A^T production: the host hands each core its adjacency as fp16 (exact for
0/1 values, a lossless repack done during input sharding); the device
xbar-DMA-transposes 128-column stripes straight into SBUF. X likewise
arrives fp16, padded to 128 columns with a ones column so X^T (with the
ones row the G-matmuls need) is a single xbar transpose.

Mask multiply p = u * A^T runs on two lanes (DVE tensor_tensor at 2x mode,
GPSIMD tensor_tensor) balanced by KNOBS; per head the GPSIMD-lane u's are
emitted first so that lane never starves. u = max(e1, e2*g) is a single
DVE tensor_scalar in 4x mode. Row broadcasts (g, 1/denom) are PE rank-1
matmuls (ones ⊗ row) through PSUM. Reciprocal via Act Ln -> Exp(-x).
"""

import numpy as np

B, N, F, FH, H = 8, 2048, 64, 64, 4
P = 128           # SBUF partitions
NT = N // P       # 16 m-tiles
C = 512           # matmul moving-operand chunk
NCH = N // C      # 4 chunks
GW = 66           # G row stride (64 feats + 1 ones + 1 pad)
HN = N // 2       # half row

_CACHE = {}

# tuning knobs (read at build time)
KNOBS = {
    "pool_ks": (
        (1, 3, 5, 7, 9, 11),
        (1, 3, 5, 7, 9, 11),
        (1, 3, 5, 7, 9, 11),
        (0, 2, 4, 6),
    ),  # per-head k's whose mask-mult goes to GPSIMD
    "u_bufs": 6,
    "p_bufs": 12,
    "lead": 3,
    "outp_bufs": 3,
    "agg_bufs": 5,
    "pst_bufs": 3,
}


def _build():
    import concourse.bacc as bacc
    import concourse.tile as tile
    import concourse.mybir as mybir
    from concourse.mybir import AluOpType as op, ActivationFunctionType as act

    f32 = mybir.dt.float32
    fp16 = mybir.dt.float16
    i32 = mybir.dt.int32

    nc = bacc.Bacc(
        "TRN2",
        target_bir_lowering=False,
        debug=False,
        enable_asserts=False,
        num_devices=8,
    )

    A_d = nc.dram_tensor("A", [N, N], fp16, kind="ExternalInput").ap()
    X_d = nc.dram_tensor("X", [N, P], fp16, kind="ExternalInput").ap()
    W_d = nc.dram_tensor("W16h", [H, F + 1, FH], fp16, kind="ExternalInput").ap()
    WT_d = nc.dram_tensor("WT16h", [H, FH, F + 1], fp16, kind="ExternalInput").ap()
    AV_d = nc.dram_tensor("AV16", [F, 2 * H], fp16, kind="ExternalInput").ap()
    OUT_d = nc.dram_tensor("OUT", [H, FH, N], f32, kind="ExternalOutput").ap()

    with tile.TileContext(nc) as tc:
        with (
            tc.tile_pool(name="const", bufs=1) as const,
            tc.tile_pool(name="big", bufs=1) as big,
            tc.tile_pool(name="stream", bufs=3) as stream,
            tc.tile_pool(name="head", bufs=2) as head,
            tc.tile_pool(name="outp", bufs=KNOBS["outp_bufs"]) as outp,
            tc.tile_pool(name="psagg", bufs=KNOBS["agg_bufs"], space="PSUM") as psagg,
            tc.tile_pool(name="pst", bufs=KNOBS["pst_bufs"], space="PSUM") as pst,
        ):
            # ---- constants --------------------------------------------
            iota_i = const.tile([P, P], i32)
            nc.gpsimd.iota(iota_i[:], pattern=[[1, P]], base=0, channel_multiplier=0)
            pidx_i = const.tile([P, 1], i32)
            nc.gpsimd.iota(pidx_i[:], pattern=[[0, 1]], base=0, channel_multiplier=1)
            iota_f = const.tile([P, P], f32)
            nc.vector.tensor_copy(iota_f[:], iota_i[:])
            pidx_f = const.tile([P, 1], f32)
            nc.vector.tensor_copy(pidx_f[:], pidx_i[:])
            ident = const.tile([P, P], fp16)
            nc.vector.tensor_scalar(ident[:], iota_f[:], pidx_f[:], None, op.is_equal)
            ones_row = const.tile([1, P], fp16)
            nc.vector.memset(ones_row[:], 1.0)

            # a_self / a_neigh fp16 columns straight from the host
            av_all = const.tile([F, 2 * H], fp16)
            av16 = av_all[:, 0:H]
            an16 = av_all[:, H : 2 * H]

            def emit_av_dma():
                nc.sync.dma_start(av_all[:], AV_d)

            # ---- X^T via one xbar transpose (host-padded fp16) --------
            XT_full = big.tile([P, N], fp16)

            def emit_xt_dma():
                nc.sync.dma_start_transpose(XT_full[:], X_d)

            XT16 = XT_full[0 : F + 1, :]

            # ---- A^T via direct fp16 xbar transpose -------------------
            AT_sb = big.tile([P, NT * N], fp16)

            def emit_transpose(k):
                nc.sync.dma_start_transpose(
                    AT_sb[:, k * N : (k + 1) * N], A_d[:, k * P : (k + 1) * P]
                )

            Wts = []

            def emit_wstage(h):
                W16 = head.tile([F + 1, FH], fp16, tag="W16", bufs=4,
                                name=f"W16_{h}")
                nc.sync.dma_start(W16[:], W_d[h])
                WT = head.tile([FH, F + 1], fp16, tag="WT", bufs=4,
                               name=f"WT_{h}")
                nc.sync.dma_start(WT[:], WT_d[h])
                Wts.append((W16, WT))

            def emit_setup(h):
                W16, WT = Wts[h]

                # wa = [W^T a | b^T a]  [65, 2]  (col 0 self, col 1 neigh)
                waps = pst.tile([F + 1, 2], f32, tag="t", name=f"waps_{h}")
                nc.tensor.matmul(waps[:, 0:1], WT[:], av16[:, h : h + 1],
                                 start=True, stop=True)
                nc.tensor.matmul(waps[:, 1:2], WT[:], an16[:, h : h + 1],
                                 start=True, stop=True)
                wa16 = head.tile([F + 1, 2], fp16, tag="wa16", bufs=2,
                                 name=f"wa16_{h}")
                nc.scalar.copy(wa16[:], waps[:])

                # score rows: s_self -> g_row = exp(-0.8 s), s_neigh staged
                g_row = head.tile([1, N], fp16, tag="g_row", bufs=1,
                                  name=f"g_row_{h}")
                snrow = head.tile([1, N], fp16, tag="snrow", bufs=1,
                                  name=f"snrow_{h}")
                cpy = lambda o, i: nc.scalar.copy(o, i)
                for c in range(NCH):
                    sl = slice(c * C, (c + 1) * C)
                    sps = pst.tile([1, C], f32, tag="t", name=f"sps_{h}_{c}")
                    nc.tensor.matmul(sps[:], wa16[:, 0:1], XT16[:, sl],
                                     start=True, stop=True)
                    nc.scalar.activation(g_row[:, sl], sps[:], act.Exp,
                                         scale=-0.8)
                    sps2 = pst.tile([1, C], f32, tag="t", name=f"sps2_{h}_{c}")
                    nc.tensor.matmul(sps2[:], wa16[:, 1:2], XT16[:, sl],
                                     start=True, stop=True)
                    cpy(snrow[:, sl], sps2[:])

                # g broadcast to 128 partitions: PE rank-1 + Act cast copies
                g_bc = head.tile([P, N], fp16, tag="g_bc", bufs=2,
                                 name=f"g_bc_{h}")
                for c in range(NCH):
                    sl = slice(c * C, (c + 1) * C)
                    gps = pst.tile([P, C], f32, tag="t", name=f"gps_{h}_{c}")
                    nc.tensor.matmul(gps[:], ones_row[:], g_row[:, sl],
                                     start=True, stop=True)
                    cpy(g_bc[:, sl], gps[:])

                # e1/e2 columns: PE transposes of snrow -> even fp16 columns
                # of psn (4-byte-aligned psum writes), strided exp reads
                psn = pst.tile([P, 2 * NT], fp16, tag="t", name=f"psn_{h}")
                psn3 = psn.rearrange("p (k two) -> p k two", two=2)
                for k in range(NT):
                    nc.tensor.transpose(
                        psn3[:, k, 0:1],
                        snrow[:, k * P : (k + 1) * P],
                        ident[0:1, 0:1],
                    )
                e1g = head.tile([P, NT], f32, tag="e1g", bufs=2, name=f"e1g_{h}")
                nc.scalar.activation(e1g[:], psn3[:, :, 0], act.Exp, scale=1.0)
                e2g = head.tile([P, NT], f32, tag="e2g", bufs=2, name=f"e2g_{h}")
                nc.scalar.activation(e2g[:], psn3[:, :, 0], act.Exp, scale=0.2)

                # G = [feats + b | 1] per m-tile
                G_all = head.tile([P, NT * GW], fp16, tag="G_all", bufs=2,
                                  name=f"G_all_{h}")
                G3 = G_all.rearrange("p (k w) -> p k w", w=GW)
                for halfg in range(2):
                    psG = pst.tile([P, (NT // 2) * FH], f32, tag="t",
                                   name=f"psG_{h}_{halfg}")
                    for j in range(NT // 2):
                        k = halfg * (NT // 2) + j
                        nc.tensor.matmul(
                            psG[:, j * FH : (j + 1) * FH],
                            XT16[:, k * P : (k + 1) * P],
                            W16[:],
                            start=True, stop=True,
                        )
                    nc.scalar.copy(
                        G3[:, halfg * (NT // 2) : (halfg + 1) * (NT // 2), 0:FH],
                        psG.rearrange("p (k f) -> p k f", f=FH),
                    )
                nc.vector.memset(G3[:, :, FH : FH + 1], 1.0)
                return (e1g, e2g, g_bc, G_all)

            def alloc_aggs(h):
                return [
                    psagg.tile([FH + 1, C], f32, tag="agg", name=f"agg{h}_{c}")
                    for c in range(NCH)
                ]

            def emit_u(h, st, k):
                e1g, e2g, g_bc, G_all = st
                u_t = stream.tile([P, N], fp16, tag="u", bufs=KNOBS["u_bufs"],
                                  name=f"u_{h}_{k}")
                nc.vector.tensor_scalar(
                    u_t[:], g_bc[:],
                    e2g[:, k : k + 1], e1g[:, k : k + 1],
                    op.mult, op.max,
                )
                return u_t

            def emit_mask(h, k, u_t, pool_lane):
                p_t = stream.tile([P, N], fp16, tag="p", bufs=KNOBS["p_bufs"],
                                  name=f"p_{h}_{k}")
                eng = nc.gpsimd if pool_lane else nc.vector
                eng.tensor_tensor(
                    p_t[:], u_t[:], AT_sb[:, k * N : (k + 1) * N], op.mult
                )
                return p_t

            def emit_pool_tile(h, st, k):
                # u computed straight into the p tile, mask applied in place
                e1g, e2g, g_bc, G_all = st
                p_t = stream.tile([P, N], fp16, tag="p", bufs=KNOBS["p_bufs"],
                                  name=f"p_{h}_{k}")
                nc.vector.tensor_scalar(
                    p_t[:], g_bc[:],
                    e2g[:, k : k + 1], e1g[:, k : k + 1],
                    op.mult, op.max,
                )
                nc.gpsimd.tensor_tensor(
                    p_t[:], p_t[:], AT_sb[:, k * N : (k + 1) * N], op.mult
                )
                return p_t

            def emit_aggs(h, aggs, k, p_t, first, last):
                for c in range(NCH):
                    sl = slice(c * C, (c + 1) * C)
                    nc.tensor.matmul(
                        aggs[c][:],
                        G_alls[h][:, k * GW : k * GW + FH + 1],
                        p_t[:, sl],
                        start=first, stop=last,
                    )

            ones_row32 = const.tile([1, FH + 1], f32)
            nc.vector.memset(ones_row32[:], 1.0)

            def emit_finals_rbs(h, aggs):
                rbs = head.tile([FH + 1, N], fp16, tag="rbs", bufs=2,
                                name=f"rbs_{h}")
                if h < H - 1:
                    # reciprocal via Act Ln -> Exp(-x); off the critical path
                    lnr = head.tile([1, N], f32, tag="lnr", bufs=1,
                                    name=f"lnr_{h}")
                    for c in range(NCH):
                        nc.scalar.activation(
                            lnr[:, c * C : (c + 1) * C],
                            aggs[c][FH : FH + 1, :], act.Ln,
                        )
                    rrow = head.tile([1, N], fp16, tag="rrow", bufs=1,
                                     name=f"rrow_{h}")
                    nc.scalar.activation(rrow[:], lnr[:], act.Exp, scale=-1.0)
                    for c in range(NCH):
                        sl = slice(c * C, (c + 1) * C)
                        rps = pst.tile([FH + 1, C], f32, tag="t",
                                       name=f"rps_{h}_{c}")
                        nc.tensor.matmul(
                            rps[:], ones_row[:, 0 : FH + 1], rrow[:, sl],
                            start=True, stop=True,
                        )
                        nc.scalar.copy(rbs[:, sl], rps[:])
                else:
                    # tail head: shortest chain via DVE reciprocal quarters
                    rrow32 = head.tile([1, N], f32, tag="lnr", bufs=1,
                                       name=f"rrow32_{h}")
                    for c in range(NCH):
                        sl = slice(c * C, (c + 1) * C)
                        nc.vector.reciprocal(
                            rrow32[:, sl], aggs[c][FH : FH + 1, :]
                        )
                        rps = pst.tile([FH + 1, C], f32, tag="t",
                                       name=f"rps_{h}_{c}")
                        nc.tensor.matmul(
                            rps[:], ones_row32[:], rrow32[:, sl],
                            start=True, stop=True,
                        )
                        nc.scalar.copy(rbs[:, sl], rps[:])
                return rbs

            def emit_outf_chunk(h, aggs, rbs, c):
                sl = slice(c * C, (c + 1) * C)
                outf = outp.tile([FH + 1, C], f32, tag="outf",
                                 name=f"outf_{h}_{c}")
                nc.vector.scalar_tensor_tensor(
                    outf[:], aggs[c][:],
                    0.0, rbs[:, sl], op.max, op.mult,
                )
                nc.scalar.dma_start(OUT_d[h, :, sl], outf[0:FH, :])

            # ---- schedule ---------------------------------------------
            emit_wstage(0)
            emit_av_dma()
            emit_xt_dma()
            for k in range(5):
                emit_transpose(k)
            for h in range(1, H):
                emit_wstage(h)
            for k in range(5, NT):
                emit_transpose(k)

            sts = [None] * H
            aggs_h = [None] * H
            G_alls = [None] * H
            sts[0] = emit_setup(0)
            G_alls[0] = sts[0][3]
            lead = KNOBS["lead"]
            carry = None  # (head, aggs, rbs, next chunk) pending outf work
            for h in range(H):
                if h + 1 < H and sts[h + 1] is None:
                    sts[h + 1] = emit_setup(h + 1)
                    G_alls[h + 1] = sts[h + 1][3]
                aggs_h[h] = alloc_aggs(h)
                pool_ks = KNOBS["pool_ks"][h]
                n_aggs = 0
                n_total = NT
                pend = []
                pool_ps = []
                pool_next = 0
                # GPSIMD-lane tiles first so that lane never starves
                for k in pool_ks:
                    pool_ps.append((k, emit_pool_tile(h, sts[h], k)))
                    if carry is not None:
                        ch, caggs, crbs, cc = carry
                        emit_outf_chunk(ch, caggs, crbs, cc)
                        carry = (ch, caggs, crbs, cc + 1) if cc + 1 < NCH else None
                if carry is not None:
                    ch, caggs, crbs, cc = carry
                    for c in range(cc, NCH):
                        emit_outf_chunk(ch, caggs, crbs, c)
                    carry = None

                def emit_one_agg(kk, pp):
                    nonlocal n_aggs
                    emit_aggs(h, aggs_h[h], kk, pp, n_aggs == 0,
                              n_aggs == n_total - 1)
                    n_aggs += 1

                dve_ks = [k for k in range(NT) if k not in pool_ks]
                for i, k in enumerate(dve_ks):
                    u_t = emit_u(h, sts[h], k)
                    pend.append((k, emit_mask(h, k, u_t, False)))
                    if len(pend) > lead:
                        emit_one_agg(*pend.pop(0))
                    # pool p's trickle in at ~1 per 2.5 DVE-lane tiles
                    if i % 2 == 1 and pool_next < len(pool_ps) and i >= 3:
                        emit_one_agg(*pool_ps[pool_next])
                        pool_next += 1
                for kk, pp in pend:
                    emit_one_agg(kk, pp)
                for kk, pp in pool_ps[pool_next:]:
                    emit_one_agg(kk, pp)
                rbs = emit_finals_rbs(h, aggs_h[h])
                if h + 1 < H:
                    carry = (h, aggs_h[h], rbs, 0)
                else:
                    for c in range(NCH):
                        emit_outf_chunk(h, aggs_h[h], rbs, c)

    nc.compile()
    return nc


def _get_nc():
    if "nc" not in _CACHE:
        _CACHE["nc"] = _build()
    return _CACHE["nc"]


def make_in_maps(inputs):
    Xf = np.asarray(inputs["X"])
    X = np.zeros((B, N, P), dtype=np.float16)
    X[:, :, 0:F] = Xf.astype(np.float16)
    X[:, :, F] = 1.0
    A = np.asarray(inputs["A"])
    W = np.asarray(inputs["W"]).astype(np.float16)
    b = np.asarray(inputs["b"]).astype(np.float16)
    a_self = np.asarray(inputs["a_self"]).astype(np.float16)
    a_neigh = np.asarray(inputs["a_neigh"]).astype(np.float16)
    W16h = np.concatenate([W, b[:, None, :]], axis=1)          # [H, F+1, FH]
    WT16h = np.ascontiguousarray(W16h.transpose(0, 2, 1))      # [H, FH, F+1]
    AV16 = np.concatenate([a_self.T, a_neigh.T], axis=1)       # [F, 2H]
    return [
        {
            # adjacency is 0/1: fp16 repack is exact (input marshaling)
            "A": np.ascontiguousarray(A[i], dtype=np.float16),
            "X": np.ascontiguousarray(X[i]),
            "W16h": np.ascontiguousarray(W16h),
            "WT16h": WT16h,
            "AV16": np.ascontiguousarray(AV16),
        }
        for i in range(B)
    ]


def run(inputs, trace=False):
    from concourse import bass_utils

    nc = _get_nc()
    in_maps = make_in_maps(inputs)
    res = bass_utils.run_bass_kernel_spmd(
        nc, in_maps, core_ids=list(range(B)), trace=trace
    )
    out = np.empty((B, N, H * FH), dtype=np.float32)
    for i in range(B):
        o = res.results[i]["OUT"]  # [H, FH, N]
        out[i] = o.transpose(2, 0, 1).reshape(N, H * FH)
    return out, res


def kernel(**inputs):
    out, _ = run(inputs, trace=False)
    return out
